# revision 9
# baseline (speedup 1.0000x reference)
"""Trainium2 Bass kernel for nn_LocalRelationalLayer_18262200943220.

The reference LocalRelationalLayer builds key/query maps and a softmax
composability tensor, but multiplies them into a feature map `fm` that is
identically zero (faithful to the torch original, see reference comment).
Everything upstream of the final 1x1x1 conv is therefore multiplied by
zero: out = einsum(zeros, f_w) + f_b == broadcast(f_b).

So the exact output is f_b broadcast to [1, 256, 14, 14, 128], bitwise
equal to the reference. The kernel shards the 256 output channels across
the 8 NeuronCores (32 channels each, replicated over 4 partitions each so
all 128 SBUF/DMA partitions carry data).

Per-core program: a single DRAM->DRAM DMA. The host pre-replicates each
core's 32 bias values into a [128, 128] f32 input block (each partition
row = its channel's bias, 512 B contiguous); the DMA reads that block with
a stride-0 middle dim ([128, 49, 128] view) and writes the whole
[128, 6272] output. 512 B descriptors stay at full DMA bus rate (>=512 B),
so the transfer runs at the modeled 360 B/ns aggregate: 3.21 MB -> 8920 ns
wire.

The default Bass preamble (per-engine register init, four const-AP
memsets with no readers, and an all-engine barrier) exists to support
multi-engine kernels with SBUF state. This program is a single SP-issued
DMA with no SBUF use and no cross-engine hazards, so none of that
scaffolding is needed for correctness: _build_bass() strips every
pre-DMA instruction except the dummy InstCall (which populates the DMA
table). The stripped program passes neuronxcc's birverifier and runs
bit-exact on the BIR simulator. What remains is irreducible: SEQ decode
(25) + HWDGE generation (625) + DGE-start delay (650) + wire (8920) +
the compiler-mandated completion-semaphore propagation (900) = 11120 ns
per TimelineSim (vs 15262 ns for the original SBUF-staged version; the
un-stripped single-DMA version is 12041 ns).
"""

import numpy as np

import concourse.bass as bass
import concourse.mybir as mybir
from concourse.bass_utils import run_bass_kernel_spmd

OUT_SHAPE = (1, 256, 14, 14, 128)  # [B, outC, 2K, 2K, 2D] from the reference
OUTC = 256
SPATIAL = 14 * 14 * 128  # 25088 voxels per output channel
N_CORES = 8
CPC = OUTC // N_CORES  # 32 channels per core
P = 128  # SBUF/DMA partitions
REP = P // CPC  # 4 partitions per channel
COLS = SPATIAL // REP  # 6272 f32 per partition row
W = 128  # source block width: 128 f32 = 512 B descriptors (full bus rate)
NREP = COLS // W  # 49 stride-0 replications per partition row

_CACHE = {}


def _build_bass():
    """Per-core graph: one DRAM->DRAM DMA, out[p, r*W+j] = fb[p, j].

    Raw Bass, no Block wrapper: the program is a single SP-issued DMACopy
    with no data hazards (source is an ExternalInput already in DRAM), so
    it needs no semaphores for ordering, no SBUF tiles, and no
    cross-engine barrier.
    """
    f32 = mybir.dt.float32
    nc = bass.Bass("TRN2", debug=False, monotonic_sem_count=0)
    fb_in = nc.dram_tensor("fb", [P, W], f32, kind="ExternalInput")
    out = nc.dram_tensor("out", [P, COLS], f32, kind="ExternalOutput")
    sem = nc.semaphore("dma_sem").__enter__()
    out_v = out.ap()[:, :].rearrange("p (r w) -> p r w", w=W)
    d = nc.sync.dma_start(out=out_v, in_=fb_in[:, None, :].broadcast_to([P, NREP, W]))
    # walrus codegen requires a sync UPDATE on DGE DMAs (sync::Update
    # front() assert); nothing in-program consumes it, so the only cost is
    # the completion-sem propagation tail after the transfer.
    d.then_inc(sem, 16)

    # Strip the default preamble: this program uses no SBUF, no const APs,
    # and only the SP engine, so the per-engine register init, the four
    # never-read const-AP memsets, and the all-engine barrier contribute
    # nothing to correctness (verified bit-exact through birverifier + BIR
    # simulation with and without them). Keep the leading dummy InstCall --
    # it populates call_to_physical_memlocs for the DMA table -- and
    # everything from our first emitted instruction (the DMACopy) onward.
    # Fail-safe: if the module layout ever differs from what this expects,
    # keep the full program (still correct, ~900 ns slower) over crashing.
    try:
        insts = nc.m.functions[0].blocks[0].instructions
        first_dma = next(
            i for i, ins in enumerate(insts) if isinstance(ins, mybir.InstDMACopy)
        )
        stripped = [
            ins for ins in insts[:first_dma] if isinstance(ins, mybir.InstCall)
        ] + insts[first_dma:]
        if any(isinstance(ins, mybir.InstCall) for ins in stripped):
            insts[:] = stripped
    except Exception:
        pass
    return nc


# Stashed BassKernelResults from the most recent run (exec_time_ns etc.);
# used by the dev harness, not by grading.
LAST_RUN = None


def kernel(**inputs) -> np.ndarray:
    global LAST_RUN
    f_b = np.ascontiguousarray(np.asarray(inputs["f_b"]), dtype=np.float32)
    assert f_b.shape == (OUTC,), f_b.shape

    # Shard channels across cores; each core's [128, 128] source block has
    # partition row 4c+r filled with channel c's bias value.
    in_maps = []
    for ci in range(N_CORES):
        shard = f_b[ci * CPC : (ci + 1) * CPC]
        col = np.repeat(shard, REP).reshape(P, 1)
        in_maps.append({"fb": np.ascontiguousarray(np.broadcast_to(col, (P, W)))})

    if "nc" not in _CACHE:
        _CACHE["nc"] = _build_bass()
    res = run_bass_kernel_spmd(_CACHE["nc"], in_maps, core_ids=list(range(N_CORES)))
    LAST_RUN = res

    # Unshard: per-core [128, 6272] -> [32, 25088]; concat channel blocks.
    parts = [np.asarray(r["out"]).reshape(CPC, SPATIAL) for r in res.results]
    return np.concatenate(parts, axis=0).reshape(OUT_SHAPE)
